# revision 2
# baseline (speedup 1.0000x reference)
"""Trainium2 Bass kernel for nn_HLALayer (higher-order linear attention).

Chunked closed form of the scan (chunk L, per head):
    N = K Q^T
    A[t,r] = sum_{s<=min(t,r)} N[s,t] N[s,r] + (Q S0 Q^T)[t,r],  kept r<=t
    O = Q D0 + A V
    S1 = S0 + K^T K ;  D1 = D0 + S0 (Q^T V) + K^T (triu(N) V)
bf16 matmuls with fp32 PSUM accumulation; all 4 local heads batched per
PSUM bank so each bank is evacuated by a single DVE/ACT op.  x is shipped
pre-transposed (and bf16) from the host, so no PE transposes of x.

Sharding: 8 cores = (batch b in {0,1}) x (head-group g in {0..3}, 4 heads
each); partial [T, D] outputs summed on host per batch.  A f32r fallback
implementation (the previous validated kernel) is embedded and used if the
primary path fails at runtime.
"""

import numpy as np
import sys

sys.path.insert(0, "/opt/trn_rl_repo")

import concourse.bacc as bacc
import concourse.mybir as mybir
from concourse.bass_utils import run_bass_kernel_spmd
from concourse.tile import TileContext

F32 = mybir.dt.float32
F32R = mybir.dt.float32r
BF16 = mybir.dt.bfloat16

D = 1024          # model dim
DL = 256          # per-core projection width (4 heads x 64)
DK = 64           # head dim
L = 128           # chunk length
NHL = 4           # local heads per core


def build_v2(T=2048):
    NCH = T // L
    nc = bacc.Bacc("TRN2", target_bir_lowering=False)

    xt_in = nc.declare_dram_parameter("xt", [D, T], BF16, isOutput=False)
    wq_in = nc.declare_dram_parameter("wq", [D, DL], BF16, isOutput=False)
    wk_in = nc.declare_dram_parameter("wk", [D, DL], BF16, isOutput=False)
    wv_in = nc.declare_dram_parameter("wv", [D, DL], BF16, isOutput=False)
    wo_in = nc.declare_dram_parameter("wo", [DL, D], BF16, isOutput=False)
    id_in = nc.declare_dram_parameter("ident", [128, 128], BF16, isOutput=False)
    mu_in = nc.declare_dram_parameter("mu", [128, 512], F32, isOutput=False)
    ml_in = nc.declare_dram_parameter("ml", [128, 512], F32, isOutput=False)
    out_d = nc.declare_dram_parameter("out", [T, D], BF16, isOutput=True)

    with TileContext(nc) as tc:
        with tc.tile_pool(name="const", bufs=1) as cpool, \
             tc.tile_pool(name="work", bufs=2) as work, \
             tc.tile_pool(name="spool", bufs=2) as spool, \
             tc.tile_pool(name="ppp", bufs=2, space="PSUM") as ppp, \
             tc.tile_pool(name="pbig", bufs=2, space="PSUM") as pbig, \
             tc.tile_pool(name="pA", bufs=2, space="PSUM") as pA, \
             tc.tile_pool(name="pOD", bufs=2, space="PSUM") as pOD:

            # ---- constants / weights (gpsimd=SWDGE queue) ----
            ident = cpool.tile([128, 128], BF16)
            nc.gpsimd.dma_start(out=ident[:], in_=id_in[:])
            mu = cpool.tile([128, 512], F32)
            nc.gpsimd.dma_start(out=mu[:], in_=mu_in[:])
            ml = cpool.tile([128, 512], F32)
            nc.gpsimd.dma_start(out=ml[:], in_=ml_in[:])
            id64 = ident[0:64, 0:64]

            wq_sb, wk_sb, wv_sb = [], [], []
            for j in range(8):
                wqt = cpool.tile([128, DL], BF16, name=f"wq{j}")
                nc.gpsimd.dma_start(out=wqt[:], in_=wq_in[128 * j:128 * (j + 1), :])
                wq_sb.append(wqt)
                wkt = cpool.tile([128, DL], BF16, name=f"wk{j}")
                nc.gpsimd.dma_start(out=wkt[:], in_=wk_in[128 * j:128 * (j + 1), :])
                wk_sb.append(wkt)
                wvt = cpool.tile([128, DL], BF16, name=f"wv{j}")
                nc.gpsimd.dma_start(out=wvt[:], in_=wv_in[128 * j:128 * (j + 1), :])
                wv_sb.append(wvt)
            wo_sb = []
            for m in range(2):
                wot = cpool.tile([128, D], BF16, name=f"wo{m}")
                nc.gpsimd.dma_start(out=wot[:], in_=wo_in[128 * m:128 * (m + 1), :])
                wo_sb.append(wot)

            xn = [None] * 8
            qt = kt = None
            vt = [None, None]
            S4 = D4 = Sb = Db = None  # fp32 states + bf16 casts

            for c in range(NCH):
                t0 = L * c
                p4 = c % 4
                p = c & 1

                # ---- x^T loads: [128, 512] tiles every 4 chunks ----
                if p4 == 0:
                    for j in range(8):
                        xnj = work.tile([128, 512], BF16, tag=f"xn{j}", bufs=2,
                                        name=f"xn{j}_{c}")
                        nc.sync.dma_start(
                            out=xnj[:], in_=xt_in[128 * j:128 * (j + 1),
                                                  t0:t0 + 512])
                        xn[j] = xnj

                # ---- phase 1 (every 2 chunks): QT/KT [128,512], V 2x[128,256]
                if p == 0:
                    psQT = ppp.tile([128, 512], F32, tag="pp", bufs=2,
                                    name=f"psQT_{c}")
                    psKT = ppp.tile([128, 512], F32, tag="pp", bufs=2,
                                    name=f"psKT_{c}")
                    for m in range(2):
                        for j in range(8):
                            xs = xn[j][:, 128 * p4:128 * p4 + 256]
                            nc.tensor.matmul(psQT[:, 256 * m:256 * (m + 1)],
                                             wq_sb[j][:, 128 * m:128 * (m + 1)],
                                             xs, start=(j == 0), stop=(j == 7))
                            nc.tensor.matmul(psKT[:, 256 * m:256 * (m + 1)],
                                             wk_sb[j][:, 128 * m:128 * (m + 1)],
                                             xs, start=(j == 0), stop=(j == 7))
                    qt = work.tile([128, 512], BF16, tag="qt", bufs=2,
                                   name=f"qt_{c}")
                    nc.scalar.copy(qt[:], psQT[:])
                    kt = work.tile([128, 512], BF16, tag="kt", bufs=2,
                                   name=f"kt_{c}")
                    nc.scalar.copy(kt[:], psKT[:])
                    for tt in range(2):
                        psV = ppp.tile([128, 256], F32, tag="pp", bufs=2,
                                       name=f"psV{tt}_{c}")
                        for j in range(8):
                            nc.tensor.matmul(
                                psV[:],
                                xn[j][:, 128 * (p4 + tt):128 * (p4 + tt) + 128],
                                wv_sb[j][:], start=(j == 0), stop=(j == 7))
                        vtt = work.tile([128, 256], BF16, tag=f"vt{tt}", bufs=2,
                                        name=f"vt{tt}_{c}")
                        nc.scalar.copy(vtt[:], psV[:])
                        vt[tt] = vtt

                def QTc(h):
                    return qt[64 * (h & 1):64 * (h & 1) + 64,
                              256 * (h >> 1) + 128 * p:256 * (h >> 1) + 128 * p + 128]

                def KTc(h):
                    return kt[64 * (h & 1):64 * (h & 1) + 64,
                              256 * (h >> 1) + 128 * p:256 * (h >> 1) + 128 * p + 128]

                def Vh(h):
                    return vt[p][:, 64 * h:64 * h + 64]

                # ---- phase 2: all 4 heads batched per bank ----
                psN = pA.tile([128, 512], F32, tag="pa", bufs=2, name=f"psN_{c}")
                for h in range(NHL):
                    nc.tensor.matmul(psN[:, 128 * h:128 * (h + 1)], KTc(h), QTc(h),
                                     start=True, stop=True)
                trn = work.tile([128, 512], BF16, tag="trn", bufs=2,
                                name=f"trn_{c}")
                nc.vector.tensor_mul(trn[:], psN[:], mu[:])

                psM = pA.tile([128, 512], F32, tag="pa", bufs=2, name=f"psM_{c}")
                for h in range(NHL):
                    nc.tensor.matmul(psM[:, 128 * h:128 * (h + 1)], QTc(h), KTc(h),
                                     start=True, stop=True)
                tlm = work.tile([128, 512], BF16, tag="tlm", bufs=2,
                                name=f"tlm_{c}")
                nc.vector.tensor_mul(tlm[:], psM[:], ml[:])

                if c > 0:
                    psQS = pOD.tile([128, 256], F32, tag="pod", bufs=2,
                                    name=f"psQS_{c}")
                    for h in range(NHL):
                        nc.tensor.matmul(
                            psQS[64 * (h & 1):64 * (h & 1) + 64,
                                 128 * (h >> 1):128 * (h >> 1) + 128],
                            Sb[64 * (h & 1):64 * (h & 1) + 64,
                               64 * (h >> 1):64 * (h >> 1) + 64],
                            QTc(h), start=True, stop=True)
                    qst = work.tile([128, 256], BF16, tag="qst", bufs=2,
                                    name=f"qst_{c}")
                    nc.scalar.copy(qst[:], psQS[:])

                psA4 = pA.tile([128, 512], F32, tag="pa", bufs=2, name=f"psA_{c}")
                for h in range(NHL):
                    sl = slice(128 * h, 128 * (h + 1))
                    nc.tensor.matmul(psA4[:, sl], trn[:, sl], trn[:, sl],
                                     start=True, stop=(c == 0))
                    if c > 0:
                        nc.tensor.matmul(
                            psA4[:, sl], QTc(h),
                            qst[64 * (h & 1):64 * (h & 1) + 64,
                                128 * (h >> 1):128 * (h >> 1) + 128],
                            start=False, stop=True)
                at = work.tile([128, 512], BF16, tag="at", bufs=2, name=f"at_{c}")
                nc.vector.tensor_mul(at[:], psA4[:], mu[:])

                psT = pA.tile([128, 512], BF16, tag="pa", bufs=2, name=f"psT_{c}")
                for h in range(NHL):
                    po = 64 * (h & 1)
                    idh = ident[po:po + 64, po:po + 64]
                    nc.tensor.transpose(psT[:, 64 * h:64 * h + 64], QTc(h), idh)
                    nc.tensor.transpose(psT[:, 256 + 64 * h:256 + 64 * h + 64],
                                        KTc(h), idh)
                qkn = work.tile([128, 512], BF16, tag="qkn", bufs=2,
                                name=f"qkn_{c}")
                nc.vector.tensor_copy(qkn[:], psT[:])

                def qn(h):
                    return qkn[:, 64 * h:64 * h + 64]

                def kn(h):
                    return qkn[:, 256 + 64 * h:256 + 64 * h + 64]

                # W2 = triu(N) V  via lhsT = tril-masked M
                psW = pOD.tile([128, 256], F32, tag="pod", bufs=2, name=f"psW_{c}")
                for h in range(NHL):
                    nc.tensor.matmul(psW[:, 64 * h:64 * h + 64],
                                     tlm[:, 128 * h:128 * (h + 1)], Vh(h),
                                     start=True, stop=True)
                w2 = work.tile([128, 256], BF16, tag="w2", bufs=2, name=f"w2_{c}")
                nc.scalar.copy(w2[:], psW[:])

                # dS | dC packed [S(0:128) | C(128:256)]
                psSC = pOD.tile([128, 256], F32, tag="pod", bufs=2,
                                name=f"psSC_{c}")
                for h in range(NHL):
                    po, qo = 64 * (h & 1), 64 * (h >> 1)
                    nc.tensor.matmul(psSC[po:po + 64, qo:qo + 64], kn(h), kn(h),
                                     start=True, stop=True)
                    nc.tensor.matmul(psSC[po:po + 64, 128 + qo:128 + qo + 64],
                                     qn(h), Vh(h), start=True, stop=True)
                S4n = spool.tile([128, 128], BF16, tag="S4", bufs=2, name=f"S4_{c}")
                if c > 0:
                    nc.vector.tensor_add(S4n[:], S4[:], psSC[:, 0:128])
                else:
                    nc.vector.tensor_copy(S4n[:], psSC[:, 0:128])
                Sbn = S4n
                dc = work.tile([128, 128], BF16, tag="dc", bufs=2, name=f"dc_{c}")
                nc.vector.tensor_copy(dc[:], psSC[:, 128:256])

                # O (cols 0:256) | Delta (cols 256:384)
                psOD = pOD.tile([128, 384], F32, tag="pod", bufs=2,
                                name=f"psOD_{c}")
                for h in range(NHL):
                    po, qo = 64 * (h & 1), 64 * (h >> 1)
                    osl = slice(128 * (h >> 1), 128 * (h >> 1) + 128)
                    nc.tensor.matmul(psOD[po:po + 64, osl], Vh(h),
                                     at[:, 128 * h:128 * (h + 1)],
                                     start=True, stop=(c == 0))
                    if c > 0:
                        nc.tensor.matmul(psOD[po:po + 64, osl],
                                         Db[po:po + 64, qo:qo + 64], QTc(h),
                                         start=False, stop=True)
                    nc.tensor.matmul(psOD[po:po + 64, 256 + qo:256 + qo + 64],
                                     kn(h), w2[:, 64 * h:64 * h + 64],
                                     start=True, stop=(c == 0))
                    if c > 0:
                        nc.tensor.matmul(psOD[po:po + 64, 256 + qo:256 + qo + 64],
                                         Sb[po:po + 64, qo:qo + 64],
                                         dc[po:po + 64, qo:qo + 64],
                                         start=False, stop=True)
                ot = work.tile([128, 256], BF16, tag="ot", bufs=2, name=f"ot_{c}")
                nc.scalar.copy(ot[:], psOD[:, 0:256])
                D4n = spool.tile([128, 128], BF16, tag="D4", bufs=2, name=f"D4_{c}")
                if c > 0:
                    nc.vector.tensor_add(D4n[:], D4[:], psOD[:, 256:384])
                else:
                    nc.vector.tensor_copy(D4n[:], psOD[:, 256:384])
                Dbn = D4n
                S4, D4, Sb, Db = S4n, D4n, Sbn, Dbn

                # ---- phase 3: output projection ----
                osb = work.tile([128, D], BF16, tag="osb", bufs=2,
                                name=f"osb_{c}")
                for ncol in range(2):
                    pso = pbig.tile([128, 512], F32, tag="pb", bufs=2,
                                    name=f"pso{ncol}_{c}")
                    nc.tensor.matmul(pso[:], ot[:, 0:128],
                                     wo_sb[0][:, 512 * ncol:512 * (ncol + 1)],
                                     start=True, stop=False)
                    nc.tensor.matmul(pso[:], ot[:, 128:256],
                                     wo_sb[1][:, 512 * ncol:512 * (ncol + 1)],
                                     start=False, stop=True)
                    nc.scalar.copy(osb[:, 512 * ncol:512 * (ncol + 1)], pso[:])
                nc.sync.dma_start(out=out_d[t0:t0 + 128, :], in_=osb[:])

    nc.compile()
    return nc


def _consts():
    import ml_dtypes
    p = np.arange(128)[:, None]
    f = np.arange(128)[None, :]
    triu = (f >= p).astype(np.float32)   # keep free >= part (incl diag)
    tril = (f <= p).astype(np.float32)
    return {
        "ident": np.eye(128, dtype=ml_dtypes.bfloat16),
        "mu": np.tile(triu, (1, 4)).astype(np.float32),
        "ml": np.tile(tril, (1, 4)).astype(np.float32),
    }


_NC_CACHE = {}


def get_nc_v2(T=2048):
    if T not in _NC_CACHE:
        _NC_CACHE[T] = build_v2(T)
    return _NC_CACHE[T]


def core_input_map(x, W_q, W_k, W_v, W_o, core):
    import ml_dtypes
    bf = ml_dtypes.bfloat16
    b, g = core // 4, core % 4
    im = {
        "xt": np.ascontiguousarray(np.asarray(x[b], np.float32).T).astype(bf),
        "wq": np.ascontiguousarray(W_q[:, DL * g:DL * (g + 1)]).astype(bf),
        "wk": np.ascontiguousarray(W_k[:, DL * g:DL * (g + 1)]).astype(bf),
        "wv": np.ascontiguousarray(W_v[:, DL * g:DL * (g + 1)]).astype(bf),
        "wo": np.ascontiguousarray(W_o[DL * g:DL * (g + 1), :]).astype(bf),
    }
    im.update(_consts())
    return im


def postprocess_core_out(arr):
    return np.asarray(arr, dtype=np.float32)


def _kernel_v2(x, W_q, W_k, W_v, W_o):
    T = x.shape[1]
    nc = get_nc_v2(T)
    in_maps = [core_input_map(x, W_q, W_k, W_v, W_o, c) for c in range(8)]
    res = run_bass_kernel_spmd(nc, in_maps, list(range(8)))
    out = np.zeros((2, T, D), np.float32)
    for c in range(8):
        out[c // 4] += np.asarray(res.results[c]["out"], dtype=np.float32)
    return out


# ---------------- embedded fallback (f32r, chunk 256) ----------------

LFB = 256         # fallback chunk length


def build_fallback(T=2048):
    NCH = T // LFB
    nc = bacc.Bacc("TRN2", target_bir_lowering=False)

    x_in = nc.declare_dram_parameter("x", [T, D], F32R, isOutput=False)
    wq_in = nc.declare_dram_parameter("wq", [D, DL], F32R, isOutput=False)
    wk_in = nc.declare_dram_parameter("wk", [D, DL], F32R, isOutput=False)
    wv_in = nc.declare_dram_parameter("wv", [D, DL], F32R, isOutput=False)
    wo_in = nc.declare_dram_parameter("wo", [DL, D], F32R, isOutput=False)
    id_in = nc.declare_dram_parameter("ident", [128, 128], F32R, isOutput=False)
    # masks (f = free index, p = partition index)
    mt0_in = nc.declare_dram_parameter("mt0", [128, 256], F32, isOutput=False)  # [triu|1]
    ms0_in = nc.declare_dram_parameter("ms0", [128, 256], F32, isOutput=False)  # [striu|1]
    mz1_in = nc.declare_dram_parameter("mz1", [128, 256], F32, isOutput=False)  # [0|triu]
    mtr_in = nc.declare_dram_parameter("mtr", [128, 128], F32, isOutput=False)  # triu
    mst_in = nc.declare_dram_parameter("mst", [128, 128], F32, isOutput=False)  # striu
    out_d = nc.declare_dram_parameter("out", [T, D], F32, isOutput=True)

    ncp = 0  # copy-engine round robin counter

    with TileContext(nc) as tc:
        with tc.tile_pool(name="const", bufs=1) as cpool, \
             tc.tile_pool(name="work", bufs=2) as work, \
             tc.tile_pool(name="spool", bufs=2) as spool, \
             tc.tile_pool(name="pp", bufs=2, space="PSUM") as pps:

            def cp(out_ap, in_ap):
                """plain copy, alternating DVE / ACT to balance load"""
                nonlocal ncp
                ncp += 1
                if ncp % 2 == 0:
                    nc.vector.tensor_copy(out_ap, in_ap)
                else:
                    nc.scalar.copy(out_ap, in_ap)

            # ---- constants / weights (gpsimd=SWDGE queue, keeps HWDGE free) ----
            ident = cpool.tile([128, 128], F32R)
            nc.gpsimd.dma_start(out=ident[:], in_=id_in[:])
            mt0 = cpool.tile([128, 256], F32)
            nc.gpsimd.dma_start(out=mt0[:], in_=mt0_in[:])
            ms0 = cpool.tile([128, 256], F32)
            nc.gpsimd.dma_start(out=ms0[:], in_=ms0_in[:])
            mz1 = cpool.tile([128, 256], F32)
            nc.gpsimd.dma_start(out=mz1[:], in_=mz1_in[:])
            mtr = cpool.tile([128, 128], F32)
            nc.gpsimd.dma_start(out=mtr[:], in_=mtr_in[:])
            mst = cpool.tile([128, 128], F32)
            nc.gpsimd.dma_start(out=mst[:], in_=mst_in[:])

            wq_sb, wk_sb, wv_sb = [], [], []
            for j in range(8):
                wqt = cpool.tile([128, DL], F32R, name=f"wq{j}")
                nc.gpsimd.dma_start(out=wqt[:], in_=wq_in[128 * j:128 * (j + 1), :])
                wq_sb.append(wqt)
                wkt = cpool.tile([128, DL], F32R, name=f"wk{j}")
                nc.gpsimd.dma_start(out=wkt[:], in_=wk_in[128 * j:128 * (j + 1), :])
                wk_sb.append(wkt)
                wvt = cpool.tile([128, DL], F32R, name=f"wv{j}")
                nc.gpsimd.dma_start(out=wvt[:], in_=wv_in[128 * j:128 * (j + 1), :])
                wv_sb.append(wvt)
            wo_sb = []
            for m in range(2):
                wot = cpool.tile([128, D], F32R, name=f"wo{m}")
                nc.gpsimd.dma_start(out=wot[:], in_=wo_in[128 * m:128 * (m + 1), :])
                wo_sb.append(wot)

            # per-head states
            S = [None] * NHL
            C = [None] * NHL
            G = [None] * NHL
            Dst = [None] * NHL

            for c in range(NCH):
                t0 = LFB * c
                # ---------- phase 1: x^T tiles via PE transpose ----------
                xn = []
                for bb in range(2):
                    xnb = work.tile([128, D], F32R, tag=f"xn{bb}", bufs=2,
                                    name=f"xn{bb}_{c}")
                    nc.sync.dma_start(out=xnb[:],
                                      in_=x_in[t0 + 128 * bb:t0 + 128 * (bb + 1), :])
                    xn.append(xnb)
                xt = []
                for j in range(8):
                    xtj = work.tile([128, LFB], F32R, tag=f"xt{j}", bufs=2,
                                    name=f"xt{j}_{c}")
                    for bb in range(2):
                        pstx = pps.tile([128, 128], F32R, tag="ps", bufs=3,
                                        name=f"pstx{j}{bb}_{c}")
                        nc.tensor.transpose(pstx[:], xn[bb][:, 128 * j:128 * (j + 1)],
                                            ident[:])
                        cp(xtj[:, 128 * bb:128 * (bb + 1)], pstx[:])
                    xt.append(xtj)

                # projections: QT/KT [dk-tile 128, t 256], V natural [t 128, dv 256]
                qt, kt = [], []
                for m in range(2):
                    psq = pps.tile([128, LFB], F32, tag="pb", bufs=5, name=f"psq{m}_{c}")
                    for j in range(8):
                        nc.tensor.matmul(psq[:], wq_sb[j][:, 128 * m:128 * (m + 1)],
                                         xt[j][:], start=(j == 0), stop=(j == 7))
                    qtm = work.tile([128, LFB], F32R, tag=f"qt{m}", bufs=2,
                                    name=f"qt{m}_{c}")
                    cp(qtm[:], psq[:])
                    qt.append(qtm)
                    psk = pps.tile([128, LFB], F32, tag="pb", bufs=5, name=f"psk{m}_{c}")
                    for j in range(8):
                        nc.tensor.matmul(psk[:], wk_sb[j][:, 128 * m:128 * (m + 1)],
                                         xt[j][:], start=(j == 0), stop=(j == 7))
                    ktm = work.tile([128, LFB], F32R, tag=f"kt{m}", bufs=2,
                                    name=f"kt{m}_{c}")
                    cp(ktm[:], psk[:])
                    kt.append(ktm)
                vt = []
                for bb in range(2):
                    psv = pps.tile([128, DL], F32, tag="pb", bufs=5, name=f"psv{bb}_{c}")
                    for j in range(8):
                        nc.tensor.matmul(psv[:], xt[j][:, 128 * bb:128 * (bb + 1)],
                                         wv_sb[j][:], start=(j == 0), stop=(j == 7))
                    vtb = work.tile([128, DL], F32R, tag=f"vt{bb}", bufs=2,
                                    name=f"vt{bb}_{c}")
                    cp(vtb[:], psv[:])
                    vt.append(vtb)

                # per-chunk output tiles (oT layout [dv 256 -> 2 tiles, t 256])
                ot = [work.tile([128, LFB], F32R, tag=f"ot{m}", bufs=2,
                                name=f"ot{m}_{c}") for m in range(2)]

                # ---------- phase 2: chunked scan per local head ----------
                for h in range(NHL):
                    m = h >> 1
                    po = 64 * (h & 1)
                    QTc = qt[m][po:po + 64, :]
                    KTc = kt[m][po:po + 64, :]
                    Vb = [vt[bb][:, 64 * h:64 * h + 64] for bb in range(2)]
                    idb = ident[po:po + 64, po:po + 64]

                    # N = K Q^T [s, t];  M = Q K^T [t, s]
                    psN0 = pps.tile([128, LFB], F32, tag="pb", bufs=5, name=f"psN0_{c}_{h}")
                    nc.tensor.matmul(psN0[:], KTc[:, 0:128], QTc, start=True, stop=True)
                    psN1 = pps.tile([128, LFB], F32, tag="pb", bufs=5, name=f"psN1_{c}_{h}")
                    nc.tensor.matmul(psN1[:], KTc[:, 128:256], QTc, start=True, stop=True)
                    psM0 = pps.tile([128, LFB], F32, tag="pb", bufs=5, name=f"psM0_{c}_{h}")
                    nc.tensor.matmul(psM0[:], QTc[:, 0:128], KTc, start=True, stop=True)
                    psM1 = pps.tile([128, LFB], F32, tag="pb", bufs=5, name=f"psM1_{c}_{h}")
                    nc.tensor.matmul(psM1[:], QTc[:, 128:256], KTc, start=True, stop=True)

                    triuN0 = work.tile([128, LFB], F32R, tag="triuN0", bufs=2,
                                       name=f"triuN0_{c}_{h}")
                    nc.vector.tensor_mul(triuN0[:], psN0[:], mt0[:])
                    n0sb = work.tile([128, LFB], F32R, tag="n0sb", bufs=2,
                                     name=f"n0sb_{c}_{h}")
                    cp(n0sb[:], psN0[:])
                    triuN1 = work.tile([128, 128], F32R, tag="triuN1", bufs=2,
                                       name=f"triuN1_{c}_{h}")
                    nc.vector.tensor_mul(triuN1[:], psN1[:, 128:256], mtr[:])
                    n1sb = work.tile([128, LFB], F32R, tag="n1sb", bufs=2,
                                     name=f"n1sb_{c}_{h}")
                    cp(n1sb[:], psN1[:])
                    smM0 = work.tile([128, LFB], F32R, tag="smM0", bufs=2,
                                     name=f"smM0_{c}_{h}")
                    nc.vector.tensor_mul(smM0[:], psM0[:], ms0[:])
                    smM1 = work.tile([128, 128], F32R, tag="smM1", bufs=2,
                                     name=f"smM1_{c}_{h}")
                    nc.vector.tensor_mul(smM1[:], psM1[:, 128:256], mst[:])

                    # QST = S0 @ QTc  [dk, t]
                    if c > 0:
                        psQST = pps.tile([64, LFB], F32, tag="ps", bufs=3,
                                         name=f"psQST_{c}_{h}")
                        nc.tensor.matmul(psQST[:], S[h][po:po + 64, :], QTc,
                                         start=True, stop=True)
                        qstsb = work.tile([128, LFB], F32R, tag="qst", bufs=2,
                                          name=f"qst_{c}_{h}")
                        cp(qstsb[po:po + 64, :], psQST[:])

                    # AT = PT + AqsT  [r, t]
                    psAT0 = pps.tile([128, LFB], F32, tag="pb", bufs=5,
                                     name=f"psAT0_{c}_{h}")
                    nc.tensor.matmul(psAT0[:], triuN0[:, 0:128], n0sb[:],
                                     start=True, stop=(c == 0))
                    if c > 0:
                        nc.tensor.matmul(psAT0[:], qstsb[po:po + 64, 0:128], QTc,
                                         start=False, stop=True)
                    psAT1 = pps.tile([128, LFB], F32, tag="pb", bufs=5,
                                     name=f"psAT1_{c}_{h}")
                    nc.tensor.matmul(psAT1[:], triuN0[:, 128:256], n0sb[:],
                                     start=True, stop=False)
                    nc.tensor.matmul(psAT1[:], triuN1[:], n1sb[:],
                                     start=False, stop=(c == 0))
                    if c > 0:
                        nc.tensor.matmul(psAT1[:], qstsb[po:po + 64, 128:256], QTc,
                                         start=False, stop=True)
                    at0 = work.tile([128, LFB], F32R, tag="at0", bufs=2,
                                    name=f"at0_{c}_{h}")
                    nc.vector.tensor_mul(at0[:], psAT0[:], mt0[:])
                    at1 = work.tile([128, LFB], F32R, tag="at1", bufs=2,
                                    name=f"at1_{c}_{h}")
                    nc.vector.tensor_mul(at1[:], psAT1[:], mz1[:])

                    # oT = V^T AT + (Q D0)^T   [dv, t]
                    psO = pps.tile([64, LFB], F32, tag="ps", bufs=3, name=f"psO_{c}_{h}")
                    nc.tensor.matmul(psO[:], Vb[0], at0[:], start=True, stop=False)
                    nc.tensor.matmul(psO[:], Vb[1], at1[:], start=False, stop=(c == 0))
                    if c > 0:
                        nc.tensor.matmul(psO[:], Dst[h][po:po + 64, :], QTc, start=False, stop=True)
                    cp(ot[m][po:po + 64, :], psO[:])

                    # natural-layout Q, K via PE transpose
                    qn, kn = [], []
                    for bb in range(2):
                        psq_t = pps.tile([128, 64], F32R, tag="ps", bufs=3,
                                         name=f"psqn{bb}_{c}_{h}")
                        nc.tensor.transpose(psq_t[:], QTc[:, 128 * bb:128 * (bb + 1)], idb)
                        qnb = work.tile([128, 64], F32R, tag=f"qn{bb}", bufs=2,
                                        name=f"qn{bb}_{c}_{h}")
                        cp(qnb[:], psq_t[:])
                        qn.append(qnb)
                        psk_t = pps.tile([128, 64], F32R, tag="ps", bufs=3,
                                         name=f"pskn{bb}_{c}_{h}")
                        nc.tensor.transpose(psk_t[:], KTc[:, 128 * bb:128 * (bb + 1)], idb)
                        knb = work.tile([128, 64], F32R, tag=f"kn{bb}", bufs=2,
                                        name=f"kn{bb}_{c}_{h}")
                        cp(knb[:], psk_t[:])
                        kn.append(knb)

                    # dS = K^T K, dC = Q^T V
                    psS = pps.tile([64, 64], F32, tag="ps", bufs=3, name=f"psS_{c}_{h}")
                    nc.tensor.matmul(psS[:], kn[0][:], kn[0][:], start=True, stop=False)
                    nc.tensor.matmul(psS[:], kn[1][:], kn[1][:], start=False, stop=True)
                    psC = pps.tile([64, 64], F32, tag="ps", bufs=3, name=f"psC_{c}_{h}")
                    nc.tensor.matmul(psC[:], qn[0][:], Vb[0], start=True, stop=False)
                    nc.tensor.matmul(psC[:], qn[1][:], Vb[1], start=False, stop=True)

                    # W2 = stril(N) V  [s, dv]
                    psW0 = pps.tile([128, 64], F32, tag="ps", bufs=3,
                                    name=f"psW0_{c}_{h}")
                    nc.tensor.matmul(psW0[:], smM0[:, 0:128], Vb[0], start=True, stop=True)
                    psW1 = pps.tile([128, 64], F32, tag="ps", bufs=3,
                                    name=f"psW1_{c}_{h}")
                    nc.tensor.matmul(psW1[:], smM0[:, 128:256], Vb[0], start=True, stop=False)
                    nc.tensor.matmul(psW1[:], smM1[:], Vb[1], start=False, stop=True)
                    w0sb = work.tile([128, 64], F32R, tag="w0sb", bufs=2,
                                     name=f"w0sb_{c}_{h}")
                    cp(w0sb[:], psW0[:])
                    w1sb = work.tile([128, 64], F32R, tag="w1sb", bufs=2,
                                     name=f"w1sb_{c}_{h}")
                    cp(w1sb[:], psW1[:])

                    # G update: Gamma = K^T W2 (+ dS C0)
                    psG = pps.tile([64, 64], F32, tag="ps", bufs=3, name=f"psG_{c}_{h}")
                    nc.tensor.matmul(psG[:], kn[0][:], w0sb[:], start=True, stop=False)
                    nc.tensor.matmul(psG[:], kn[1][:], w1sb[:], start=False, stop=(c == 0))
                    if c > 0:
                        dssb = work.tile([128, 64], F32R, tag="dssb", bufs=2,
                                         name=f"dssb_{c}_{h}")
                        cp(dssb[po:po + 64, :], psS[:])
                        nc.tensor.matmul(psG[:], dssb[po:po + 64, :], C[h][po:po + 64, :],
                                         start=False, stop=True)

                    # new states
                    Snew = spool.tile([128, 64], F32R, tag=f"S{h}", bufs=2,
                                      name=f"S{h}_{c}")
                    Cnew = spool.tile([128, 64], F32R, tag=f"C{h}", bufs=2,
                                      name=f"C{h}_{c}")
                    Gnew = spool.tile([128, 64], F32R, tag=f"G{h}", bufs=2,
                                      name=f"G{h}_{c}")
                    if c > 0:
                        nc.vector.tensor_add(Snew[po:po + 64, :], S[h][po:po + 64, :], psS[:])
                        nc.vector.tensor_add(Cnew[po:po + 64, :], C[h][po:po + 64, :], psC[:])
                        nc.vector.tensor_add(Gnew[po:po + 64, :], G[h][po:po + 64, :], psG[:])
                    else:
                        nc.vector.tensor_copy(Snew[po:po + 64, :], psS[:])
                        nc.vector.tensor_copy(Cnew[po:po + 64, :], psC[:])
                        nc.vector.tensor_copy(Gnew[po:po + 64, :], psG[:])
                    psD = pps.tile([64, 64], F32, tag="ps", bufs=3, name=f"psD_{c}_{h}")
                    nc.tensor.matmul(psD[:], Snew[po:po + 64, :], Cnew[po:po + 64, :],
                                     start=True, stop=True)
                    Dnew = spool.tile([128, 64], F32R, tag=f"D{h}", bufs=2,
                                      name=f"D{h}_{c}")
                    nc.vector.tensor_sub(Dnew[po:po + 64, :], psD[:], Gnew[po:po + 64, :])
                    S[h], C[h], G[h], Dst[h] = Snew, Cnew, Gnew, Dnew

                # ---------- phase 3: output projection for this chunk ----------
                for bb in range(2):
                    for ncol in range(2):
                        pso = pps.tile([128, 512], F32, tag="pb", bufs=5,
                                       name=f"pso{bb}{ncol}_{c}")
                        nc.tensor.matmul(pso[:], ot[0][:, 128 * bb:128 * (bb + 1)],
                                         wo_sb[0][:, 512 * ncol:512 * (ncol + 1)],
                                         start=True, stop=False)
                        nc.tensor.matmul(pso[:], ot[1][:, 128 * bb:128 * (bb + 1)],
                                         wo_sb[1][:, 512 * ncol:512 * (ncol + 1)],
                                         start=False, stop=True)
                        osb = work.tile([128, 512], F32, tag="osb", bufs=3,
                                        name=f"osb{bb}{ncol}_{c}")
                        cp(osb[:], pso[:])
                        nc.scalar.dma_start(
                            out=out_d[t0 + 128 * bb:t0 + 128 * (bb + 1),
                                      512 * ncol:512 * (ncol + 1)],
                            in_=osb[:])

    nc.compile()
    return nc


def _masks_fb():
    p = np.arange(128)[:, None]
    f = np.arange(128)[None, :]
    triu = (f >= p).astype(np.float32)
    striu = (f > p).astype(np.float32)
    ones = np.ones((128, 128), np.float32)
    zeros = np.zeros((128, 128), np.float32)
    return {
        "ident": np.eye(128, dtype=np.float32),
        "mt0": np.concatenate([triu, ones], axis=1),
        "ms0": np.concatenate([striu, ones], axis=1),
        "mz1": np.concatenate([zeros, triu], axis=1),
        "mtr": triu,
        "mst": striu,
    }


_NC_CACHE_FB = {}


def get_nc_fallback(T=2048):
    if T not in _NC_CACHE:
        _NC_CACHE_FB[T] = build_fallback(T)
    return _NC_CACHE_FB[T]


def _kernel_fallback(x, W_q, W_k, W_v, W_o):
    T = x.shape[1]
    nc = get_nc_fallback(T)
    masks = _masks_fb()
    in_maps = []
    for c in range(8):
        b, g = c // 4, c % 4
        im = {
            "x": np.ascontiguousarray(x[b]).astype(np.float32),
            "wq": np.ascontiguousarray(W_q[:, DL * g:DL * (g + 1)]).astype(np.float32),
            "wk": np.ascontiguousarray(W_k[:, DL * g:DL * (g + 1)]).astype(np.float32),
            "wv": np.ascontiguousarray(W_v[:, DL * g:DL * (g + 1)]).astype(np.float32),
            "wo": np.ascontiguousarray(W_o[DL * g:DL * (g + 1), :]).astype(np.float32),
        }
        im.update(masks)
        in_maps.append(im)
    res = run_bass_kernel_spmd(nc, in_maps, list(range(8)))
    out = np.zeros((2, T, D), np.float32)
    for c in range(8):
        out[c // 4] += res.results[c]["out"]
    return out



def get_nc(T=2048):
    """Primary graph (for test harness timeline inspection)."""
    return get_nc_v2(T)


def kernel(x, W_q, W_k, W_v, W_o):
    try:
        return _kernel_v2(x, W_q, W_k, W_v, W_o)
    except Exception:
        return _kernel_fallback(x, W_q, W_k, W_v, W_o)
